# revision 13
# baseline (speedup 1.0000x reference)
"""Trainium2 Bass kernel for a 2-layer GCN discriminator (GCNConv -> sigmoid) x2.

Strategy
--------
With N=4096 nodes and E=262144 edges (avg degree 64), the gather/segment-sum
message passing is densified: the normalized adjacency
A[d, s] = sum_{edges (s,d)} dinv[s]*dinv[d]  (+ dinv[i]^2 self loops)
is built on the host as a dense 4096x4096 matrix.  The whole network is then

    x1  = sigmoid(A @ (x @ W1) + b1)
    out = sigmoid(A @ (x1 @ W2) + b2)

i.e. two dense 4096^3 GEMMs plus small epilogues -- ideal for the PE array.

Sharding over 8 cores: both layers are column-sharded (each core computes 512
columns of H = x@W1 and of x1), the tiny layer-2 contraction h2 = x1 @ W2 is
partial per-core and summed across cores, and the final out rows are
row-sharded (512 rows per core).

GEMM1 keeps the classic layout (x^T panels stationary, W1 columns moving).
GEMM2 is operand-swapped: the H tiles (produced by GEMM1 with s on the
partition axis) are the stationary operand and 1024-wide panels of A^T are
the moving operand, so each DoubleRow matmul streams 2048 rows per weight
load (vs 1024 in the naive layout) -- the LdWeights overhead drops 4x.  The
swapped output x1^T [j, m] also lets b1 ride the activation bias (per
partition) and the x1@W2 partial ride DVE (per-partition scalar multiply) +
a GpSimd partition-reduce, entirely off the Tensor engine.

The cross-core sum of the x1@W2 partials runs as four small (4 KB) AllReduce
collectives, one per 1024-column chunk of GEMM2, so all but the last overlap
GEMM2 compute; the final-stage matmuls for chunk c are issued after GEMM2's
chunk c+1 so the tensor queue never stalls on a collective until the tail.

The two big GEMMs run in fp8-e4m3 with DoubleRow perf mode (2 contraction
rows per PE cell per cycle).  W1 and A are pre-scaled by 64 on the host so
their entries sit in e4m3's normal range; the scale is removed for free in
the activation epilogues (ACT computes func(in*scale + bias)).  PSUM
accumulation is fp32 throughout; the final stage contracts h2 (fp8) against
the core's own 512 columns of 64*A^T (fp8, DoubleRow).
"""

import numpy as np
import ml_dtypes

N = 4096
E = 262144
P = 128
NCORES = 8
JC = N // NCORES          # 512 feature-cols (layer1/x1) / out-rows per core
KT = N // P               # 32 contraction tiles
MT = N // P               # 32 row tiles
MW = 1024                 # GEMM2 output-column chunk width
NMH = N // MW             # 4 chunks
NJT = JC // P             # 4 j-tiles per core
WSCALE = 64.0             # host pre-scale on W1 and A for fp8 range

_BF16 = ml_dtypes.bfloat16
_FP8 = ml_dtypes.float8_e4m3

_CACHE = {}


def _build_bass_program(with_b1=True):
    """Build + compile the SPMD Bass program (identical on all 8 cores)."""
    import concourse.bass as bass  # noqa: F401
    import concourse.bacc as bacc
    import concourse.tile as tile
    import concourse.mybir as mybir
    from concourse import bass_isa
    from concourse.bass_interp import get_hw_module

    dt = mybir.dt
    AF = mybir.ActivationFunctionType
    DR = mybir.MatmulPerfMode.DoubleRow
    ALU = mybir.AluOpType

    nc = bacc.Bacc("TRN2", target_bir_lowering=False, debug=False,
                   num_devices=NCORES)

    # ---- kernel I/O (per-core) ----
    # xp_t[m, p, t*128+c] = x[m*128+c, t*128+p]      (pre-tiled lhsT panels)
    xp_t = nc.dram_tensor("xp_t", [MT, P, N], dt.float8e4, kind="ExternalInput")
    # w1_t[p, t*512+j] = 64*W1[t*128+p, c*JC+j]
    w1_t = nc.dram_tensor("w1_t", [P, KT * JC], dt.float8e4, kind="ExternalInput")
    # at2_t[p, t*4096+m] = 64*AT[t*128+p, m]         (GEMM2 moving panels)
    at2_t = nc.dram_tensor("at2_t", [P, KT * N], dt.float8e4, kind="ExternalInput")
    # atf_t[p, t*512+r] = 64*AT[t*128+p, c*JC+r]     (final-stage rhs slice)
    atf_t = nc.dram_tensor("atf_t", [P, KT * JC], dt.float8e4, kind="ExternalInput")
    # w2c[p, jt] = W2[c*JC + jt*128 + p]
    w2c = nc.dram_tensor("w2c", [P, NJT], dt.float32, kind="ExternalInput")
    b1f = (nc.dram_tensor("b1f", [P, NJT], dt.float32, kind="ExternalInput")
           if with_b1 else None)  # raw b1 (bias rides the ACT epilogue)
    b2v = nc.dram_tensor("b2v", [1, 1], dt.float32, kind="ExternalInput")
    outc = nc.dram_tensor("outc", [1, JC], dt.float32, kind="ExternalOutput")

    with tile.TileContext(nc) as tc:
        with tc.tile_pool(name="const", bufs=1) as const, \
             tc.tile_pool(name="xpool", bufs=3) as xpool, \
             tc.tile_pool(name="s1pool", bufs=2) as s1pool, \
             tc.tile_pool(name="pspool", bufs=2, space="PSUM") as pspool, \
             tc.tile_pool(name="ps2pool", bufs=2, space="PSUM") as ps2pool, \
             tc.tile_pool(name="psfinal", bufs=1, space="PSUM") as psfinal, \
             tc.tile_pool(name="drampool", bufs=1, space="DRAM") as drampool:

            # ---- resident SBUF tensors ----
            w1_sb = const.tile([P, KT, JC], dt.float8e4)
            at_sb = const.tile([P, KT, N], dt.float8e4)
            atf_sb = const.tile([P, KT, JC], dt.float8e4)
            h_sb = const.tile([P, MT, JC], dt.float8e4)
            w2_sb = const.tile([P, NJT], dt.float32)
            b1_sb = const.tile([P, NJT], dt.float32) if with_b1 else None
            b2_sb = const.tile([1, 1], dt.float32)
            acc_sb = const.tile([P, MW], dt.float32)
            h2g_sb = const.tile([P, KT], dt.float32)
            # inner dim padded to 16 B: DR ldweights needs k-pair step %16==0
            h2b_sb = const.tile([P, KT, 16], dt.float8e4)
            o_sb = const.tile([1, JC], dt.float32)

            # DRAM scratch for the cross-core h2 reduction (4 chunks)
            p2d = []
            h2d = []
            for i in range(NMH):
                p2d_i = drampool.tile([1, MW], dt.float32, tag=f"p2d{i}",
                                      name=f"p2d{i}")
                h2d_i = drampool.tile([1, MW], dt.float32, addr_space="Shared",
                                      tag=f"h2d{i}", name=f"h2d{i}")
                p2d.append(p2d_i)
                h2d.append(h2d_i)

            # First matmul needs xp[0] + the first w1 k-tiles; order the sync
            # queue so m=0's k-chain is paced by the w1 stream, with xp panels
            # slotted where they're needed.
            xp0 = xpool.tile([P, KT, P], dt.float8e4, tag="xp")
            nc.sync.dma_start(
                w1_sb[:, 0:2, :],
                w1_t.ap()[:, 0:2 * JC].rearrange("p (t j) -> p t j", j=JC))
            nc.sync.dma_start(xp0[:], xp_t.ap()[0].rearrange("p (t c) -> p t c", c=P))
            W1CH = 10
            for k in range(2, KT, W1CH):
                ke = min(k + W1CH, KT)
                nc.sync.dma_start(
                    w1_sb[:, k:ke, :],
                    w1_t.ap()[:, k * JC:ke * JC].rearrange(
                        "p (t j) -> p t j", j=JC))
            nc.gpsimd.dma_start(w2_sb[:], w2c.ap())
            if with_b1:
                nc.gpsimd.dma_start(b1_sb[:], b1f.ap())
            nc.gpsimd.dma_start(b2_sb[:], b2v.ap())
            # GEMM2 moving panels + final rhs stream on the gpsimd queue
            # during GEMM1 (2 MB chunks of 4 k-tiles).
            for t0 in range(0, KT, 4):
                nc.gpsimd.dma_start(
                    at_sb[:, t0:t0 + 4, :],
                    at2_t.ap()[:, t0 * N:(t0 + 4) * N].rearrange(
                        "p (t n) -> p t n", n=N))
            nc.gpsimd.dma_start(
                atf_sb[:], atf_t.ap().rearrange("p (t j) -> p t j", j=JC))

            # ---- GEMM 1 (fp8 DoubleRow): 64*H[:, Cc] = x @ (64*W1[:, Cc]) ----
            for m in range(MT):
                if m == 0:
                    xp = xp0
                else:
                    xp = xpool.tile([P, KT, P], dt.float8e4, tag="xp")
                    nc.sync.dma_start(
                        xp[:], xp_t.ap()[m].rearrange("p (t c) -> p t c", c=P))
                ps1 = pspool.tile([P, JC], dt.float32, tag="ps1")
                for k in range(0, KT, 2):
                    nc.tensor.matmul(
                        ps1[:],
                        xp[:, k:k + 2, :],
                        w1_sb[:, k:k + 2, :],
                        start=(k == 0),
                        stop=(k == KT - 2),
                        perf_mode=DR,
                    )
                # PSUM -> SBUF: H = (64H)/64, cast to fp8 (stationary of GEMM 2)
                nc.scalar.mul(h_sb[:, m, :], ps1[:], 1.0 / WSCALE)

            # ---- GEMM 2 (fp8 DoubleRow, swapped): for each 1024-col chunk mh
            #      64*x1^T[jt, m'] = H^T @ (64*AT[:, m']) ; x1 = sigmoid(/64+b1)
            #      acc[j, m'] = x1^T * W2[j] ; p2 = partition-reduce(acc)
            #      h2 chunk = AllReduce(p2) ; finals(mh) issued one chunk late.
            def finals(c):
                for i in range(NJT):
                    a = 8 * c + 2 * i
                    nc.tensor.matmul(
                        ps3[:],
                        h2b_sb[:, a:a + 2, 0:1],
                        atf_sb[:, a:a + 2, :],
                        start=(c == 0 and i == 0),
                        stop=(c == NMH - 1 and i == NJT - 1),
                        perf_mode=DR,
                    )

            ps3 = psfinal.tile([1, JC], dt.float32, tag="ps3")
            for mh in range(NMH):
                for j in range(NJT):
                    # two 512-col output chunks share each stationary load
                    ps2a = ps2pool.tile([P, MW // 2], dt.float32, tag="ps2a")
                    ps2b = ps2pool.tile([P, MW // 2], dt.float32, tag="ps2b")
                    for k in range(0, KT, 2):
                        lhsT = h_sb[:, k:k + 2, j * P:(j + 1) * P]
                        for mq, ps2 in ((0, ps2a), (1, ps2b)):
                            m0 = mh * MW + mq * (MW // 2)
                            nc.tensor.matmul(
                                ps2[:],
                                lhsT,
                                at_sb[:, k:k + 2, m0:m0 + MW // 2],
                                start=(k == 0),
                                stop=(k == KT - 2),
                                perf_mode=DR,
                            )
                    s1 = s1pool.tile([P, MW], dt.bfloat16, tag="s1")
                    for mq, ps2 in ((0, ps2a), (1, ps2b)):
                        sl = s1[:, mq * (MW // 2):(mq + 1) * (MW // 2)]
                        if with_b1:
                            nc.scalar.activation(sl, ps2[:], AF.Sigmoid,
                                                 scale=1.0 / WSCALE,
                                                 bias=b1_sb[:, j:j + 1])
                        else:
                            nc.scalar.activation(sl, ps2[:], AF.Sigmoid,
                                                 scale=1.0 / WSCALE)
                    if j == 0:
                        nc.vector.tensor_scalar(
                            out=acc_sb[:], in0=s1[:],
                            scalar1=w2_sb[:, 0:1], scalar2=None,
                            op0=ALU.mult)
                    else:
                        nc.vector.scalar_tensor_tensor(
                            out=acc_sb[:], in0=s1[:],
                            scalar=w2_sb[:, j:j + 1], in1=acc_sb[:],
                            op0=ALU.mult, op1=ALU.add)
                # p2 chunk = sum over the 128 j-partitions (all rows get it)
                nc.gpsimd.partition_all_reduce(
                    acc_sb[:], acc_sb[:], P, bass_isa.ReduceOp.add)
                nc.gpsimd.dma_start(p2d[mh][:], acc_sb[0:1, :])
                nc.gpsimd.collective_compute(
                    "AllReduce", ALU.add,
                    replica_groups=[list(range(NCORES))],
                    ins=[p2d[mh].opt()], outs=[h2d[mh].opt()])
                # transpose [8t, 128p] -> [128p, 8t] on the gather-in
                nc.gpsimd.dma_start(
                    h2g_sb[:, mh * 8:(mh + 1) * 8],
                    h2d[mh].rearrange("o (t p) -> p (o t)", p=P))
                nc.vector.tensor_copy(
                    out=h2b_sb[:, mh * 8:(mh + 1) * 8, 0:1],
                    in_=h2g_sb[:, mh * 8:(mh + 1) * 8])
                if mh >= 1:
                    finals(mh - 1)
            finals(NMH - 1)

            # out[Rc] = sigmoid(q/64 + b2)
            nc.scalar.activation(o_sb[:], ps3[:], AF.Sigmoid,
                                 scale=1.0 / WSCALE, bias=b2_sb[:])
            nc.gpsimd.dma_start(outc.ap(), o_sb[:])

    nc.compile()
    nc.m = get_hw_module(nc.m)
    return nc


def _host_preprocess(x, edge_index, W1, b1, W2, b2):
    """Build dense AT + pre-tiled fp8 operands; returns per-core in_maps."""
    edge_index = np.asarray(edge_index)
    src = edge_index[0].astype(np.int64)
    dst = edge_index[1].astype(np.int64)
    deg = np.bincount(dst, minlength=N).astype(np.float64) + 1.0
    dinv = 1.0 / np.sqrt(deg)
    vals = dinv[src] * dinv[dst]
    # AT[s, d] = A[d, s] (accumulates duplicate edges, like segment_sum)
    AT = np.bincount(src * N + dst, weights=vals, minlength=N * N)
    AT = AT.reshape(N, N)
    idx = np.arange(N)
    AT[idx, idx] += dinv * dinv
    AT64 = (AT * float(WSCALE)).astype(np.float32)

    x32 = np.asarray(x, dtype=np.float32)
    W1_32 = np.asarray(W1, dtype=np.float32)
    b1_32 = np.asarray(b1, dtype=np.float32)
    W2_32 = np.asarray(W2, dtype=np.float32).reshape(N)
    b2_32 = np.asarray(b2, dtype=np.float32).reshape(1)

    # xp_t[m, p, t*128+c] = x[m*128+c, t*128+p]
    xp_t = np.ascontiguousarray(
        x32.reshape(MT, P, KT, P).transpose(0, 3, 2, 1).reshape(MT, P, N)
    ).astype(_FP8)
    # at2_t[p, t*4096+m] = 64*AT[t*128+p, m]
    at2_t = np.ascontiguousarray(
        AT64.reshape(KT, P, N).transpose(1, 0, 2).reshape(P, KT * N)
    ).astype(_FP8)

    W1_s = (W1_32 * np.float32(WSCALE)).astype(_FP8)

    in_maps = []
    for c in range(NCORES):
        cols = slice(c * JC, (c + 1) * JC)
        w1_tc = np.ascontiguousarray(
            W1_s[:, cols].reshape(KT, P, JC).transpose(1, 0, 2).reshape(P, KT * JC)
        )
        atf_tc = np.ascontiguousarray(
            AT64[:, cols].reshape(KT, P, JC).transpose(1, 0, 2).reshape(P, KT * JC)
        ).astype(_FP8)
        in_maps.append({
            "xp_t": xp_t,
            "w1_t": w1_tc,
            "at2_t": at2_t,
            "atf_t": atf_tc,
            "w2c": np.ascontiguousarray(
                W2_32[cols].reshape(NJT, P).T).astype(np.float32),
            "b1f": np.ascontiguousarray(
                b1_32[cols].reshape(NJT, P).T).astype(np.float32),
            "b2v": b2_32.reshape(1, 1).astype(np.float32),
        })
    return in_maps


def kernel(x, edge_index, W1, b1, W2, b2, _trace=False, _premaps=None,
           _trace_cores=None):
    from concourse import bass_utils

    with_b1 = bool(np.any(np.asarray(b1)))
    key = f"nc_b1={with_b1}"
    if key not in _CACHE:
        _CACHE[key] = _build_bass_program(with_b1=with_b1)
    nc = _CACHE[key]

    in_maps = _premaps if _premaps is not None else _host_preprocess(
        x, edge_index, W1, b1, W2, b2)
    if not with_b1:
        in_maps = [{k: v for k, v in m.items() if k != "b1f"} for m in in_maps]

    res = bass_utils.run_bass_kernel_spmd(
        nc, in_maps, core_ids=list(range(NCORES)), trace=_trace,
        trace_cores=_trace_cores,
    )
    out = np.concatenate(
        [np.asarray(res.results[c]["outc"]).reshape(JC) for c in range(NCORES)]
    ).reshape(N, 1).astype(np.float32)
    if _trace:
        _CACHE["last_result"] = res
    return out


# revision 15
# speedup vs baseline: 1.6884x; 1.6884x over previous
"""Trainium2 Bass kernel for a 2-layer GCN discriminator (GCNConv -> sigmoid) x2.

Strategy
--------
With N=4096 nodes and E=262144 edges (avg degree 64), the gather/segment-sum
message passing is densified: the normalized adjacency
A[d, s] = sum_{edges (s,d)} dinv[s]*dinv[d]  (+ dinv[i]^2 self loops)
is built on the host as a dense 4096x4096 matrix.  The whole network is then

    x1  = sigmoid(A @ (x @ W1) + b1)
    out = sigmoid(A @ (x1 @ W2) + b2)

i.e. two dense 4096^3 GEMMs plus small epilogues -- ideal for the PE array.

Sharding over 8 cores: both layers are column-sharded (each core computes 512
columns of H = x@W1 and of x1), the tiny layer-2 contraction h2 = x1 @ W2 is
partial per-core and summed across cores, and the final out rows are
row-sharded (512 rows per core).

GEMM1 keeps the classic layout (x^T panels stationary, W1 columns moving).
GEMM2 is operand-swapped: the H tiles (produced by GEMM1 with s on the
partition axis) are the stationary operand and 1024-wide panels of A^T are
the moving operand, so each DoubleRow matmul streams 2048 rows per weight
load (vs 1024 in the naive layout) -- the LdWeights overhead drops 4x.  The
swapped output x1^T [j, m] also lets b1 ride the activation bias (per
partition) and the x1@W2 partial ride DVE (per-partition scalar multiply) +
a GpSimd partition-reduce, entirely off the Tensor engine.

The cross-core sum of the x1@W2 partials runs as four small (4 KB) AllReduce
collectives, one per 1024-column chunk of GEMM2, so all but the last overlap
GEMM2 compute; the final-stage matmuls for chunk c are issued after GEMM2's
chunk c+1 so the tensor queue never stalls on a collective until the tail.

The two big GEMMs run in fp8-e4m3 with DoubleRow perf mode (2 contraction
rows per PE cell per cycle).  W1 and A are pre-scaled by 64 on the host so
their entries sit in e4m3's normal range; the scale is removed for free in
the activation epilogues (ACT computes func(in*scale + bias)).  PSUM
accumulation is fp32 throughout; the final stage contracts h2 (fp8) against
the core's own 512 columns of 64*A^T (fp8, DoubleRow).
"""

import numpy as np
import ml_dtypes

N = 4096
E = 262144
P = 128
NCORES = 8
JC = N // NCORES          # 512 feature-cols (layer1/x1) / out-rows per core
KT = N // P               # 32 contraction tiles
MT = N // P               # 32 row tiles
MW = 1024                 # GEMM2 output-column chunk width
NMH = N // MW             # 4 chunks
NJT = JC // P             # 4 j-tiles per core
WSCALE = 64.0             # host pre-scale on W1 and A for fp8 range

_BF16 = ml_dtypes.bfloat16
_FP8 = ml_dtypes.float8_e4m3

_CACHE = {}


def _build_bass_program(with_b1=True):
    """Build + compile the SPMD Bass program (identical on all 8 cores)."""
    import concourse.bass as bass  # noqa: F401
    import concourse.bacc as bacc
    import concourse.tile as tile
    import concourse.mybir as mybir
    from concourse import bass_isa
    from concourse.bass_interp import get_hw_module

    dt = mybir.dt
    AF = mybir.ActivationFunctionType
    DR = mybir.MatmulPerfMode.DoubleRow
    ALU = mybir.AluOpType

    nc = bacc.Bacc("TRN2", target_bir_lowering=False, debug=False,
                   num_devices=NCORES)

    # ---- kernel I/O (per-core) ----
    # xp_t[m, p, t*128+c] = x[m*128+c, t*128+p]      (pre-tiled lhsT panels)
    xp_t = nc.dram_tensor("xp_t", [MT, P, N], dt.float8e4, kind="ExternalInput")
    # w1_t[p, t*512+j] = 64*W1[t*128+p, c*JC+j]
    w1_t = nc.dram_tensor("w1_t", [P, KT * JC], dt.float8e4, kind="ExternalInput")
    # at2_t[p, t*4096+m] = 64*AT[t*128+p, m]         (GEMM2 moving panels)
    at2_t = nc.dram_tensor("at2_t", [P, KT * N], dt.float8e4, kind="ExternalInput")
    # atf_t[p, t*512+r] = 64*AT[t*128+p, c*JC+r]     (final-stage rhs slice)
    atf_t = nc.dram_tensor("atf_t", [P, KT * JC], dt.float8e4, kind="ExternalInput")
    # w2c[p, jt] = W2[c*JC + jt*128 + p]
    w2c = nc.dram_tensor("w2c", [P, NJT], dt.float32, kind="ExternalInput")
    b1f = (nc.dram_tensor("b1f", [P, NJT], dt.float32, kind="ExternalInput")
           if with_b1 else None)  # raw b1 (bias rides the ACT epilogue)
    b2v = nc.dram_tensor("b2v", [1, 1], dt.float32, kind="ExternalInput")
    outc = nc.dram_tensor("outc", [1, JC], dt.float32, kind="ExternalOutput")

    with tile.TileContext(nc) as tc:
        with tc.tile_pool(name="const", bufs=1) as const, \
             tc.tile_pool(name="xpool", bufs=3) as xpool, \
             tc.tile_pool(name="s1pool", bufs=2) as s1pool, \
             tc.tile_pool(name="pspool", bufs=2, space="PSUM") as pspool, \
             tc.tile_pool(name="ps2pool", bufs=2, space="PSUM") as ps2pool, \
             tc.tile_pool(name="psfinal", bufs=1, space="PSUM") as psfinal, \
             tc.tile_pool(name="drampool", bufs=1, space="DRAM") as drampool:

            # ---- resident SBUF tensors ----
            w1_sb = const.tile([P, KT, JC], dt.float8e4)
            at_sb = const.tile([P, KT, N], dt.float8e4)
            atf_sb = const.tile([P, KT, JC], dt.float8e4)
            h_sb = const.tile([P, MT, JC], dt.float8e4)
            w2_sb = const.tile([P, NJT], dt.float32)
            b1_sb = const.tile([P, NJT], dt.float32) if with_b1 else None
            b2_sb = const.tile([1, 1], dt.float32)
            acc_sb = const.tile([P, MW], dt.float32)
            h2g_sb = const.tile([P, KT], dt.float32)
            # inner dim padded to 16 B: DR ldweights needs k-pair step %16==0
            h2b_sb = const.tile([P, KT, 16], dt.float8e4)
            o_sb = const.tile([1, JC], dt.float32)

            # DRAM scratch for the cross-core h2 reduction (4 chunks)
            p2d = []
            h2d = []
            for i in range(NMH):
                p2d_i = drampool.tile([1, MW], dt.float32, tag=f"p2d{i}",
                                      name=f"p2d{i}")
                h2d_i = drampool.tile([1, MW], dt.float32, addr_space="Shared",
                                      tag=f"h2d{i}", name=f"h2d{i}")
                p2d.append(p2d_i)
                h2d.append(h2d_i)

            # Warm up the CC stream immediately so its all-core entry barrier
            # overlaps GEMM1 instead of gating the first real AllReduce.
            ccw_in = drampool.tile([1, 8], dt.float32, tag="ccw_in",
                                   name="ccw_in")
            ccw_out = drampool.tile([1, 8], dt.float32, addr_space="Shared",
                                    tag="ccw_out", name="ccw_out")
            nc.gpsimd.collective_compute(
                "AllReduce", ALU.add,
                replica_groups=[list(range(NCORES))],
                ins=[ccw_in.opt()], outs=[ccw_out.opt()])
            nc.gpsimd.dma_start(w2_sb[:], w2c.ap())
            if with_b1:
                nc.gpsimd.dma_start(b1_sb[:], b1f.ap())
            nc.gpsimd.dma_start(b2_sb[:], b2v.ap())

            # Sync-queue order paces everything: m=0's k-chain rides the w1
            # stream; the at2/atf panels (18 MB, needed only after GEMM1) are
            # interleaved 1:1 with xp panels so they never starve the xp feed.
            xp0 = xpool.tile([P, KT, P], dt.float8e4, tag="xp")
            nc.sync.dma_start(
                w1_sb[:, 0:2, :],
                w1_t.ap()[:, 0:2 * JC].rearrange("p (t j) -> p t j", j=JC))
            nc.sync.dma_start(xp0[:], xp_t.ap()[0].rearrange("p (t c) -> p t c", c=P))
            W1CH = 10
            for k in range(2, KT, W1CH):
                ke = min(k + W1CH, KT)
                nc.sync.dma_start(
                    w1_sb[:, k:ke, :],
                    w1_t.ap()[:, k * JC:ke * JC].rearrange(
                        "p (t j) -> p t j", j=JC))

            def at2_chunk(t):
                nc.sync.dma_start(
                    at_sb[:, t, :], at2_t.ap()[:, t * N:(t + 1) * N])

            # ---- GEMM 1 (fp8 DoubleRow): 64*H[:, Cc] = x @ (64*W1[:, Cc]) ----
            # Each xp stationary feeds two 256-col matmuls so consecutive
            # matmuls share the weight load (halves LDWEIGHTS exposure).
            for m in range(MT):
                if m == 0:
                    xp = xp0
                else:
                    xp = xpool.tile([P, KT, P], dt.float8e4, tag="xp")
                    nc.sync.dma_start(
                        xp[:], xp_t.ap()[m].rearrange("p (t c) -> p t c", c=P))
                if m >= 3:
                    at2_chunk(m - 3)
                ps1 = pspool.tile([P, JC], dt.float32, tag="ps1")
                for k in range(0, KT, 2):
                    lhsT = xp[:, k:k + 2, :]
                    for jq in range(2):
                        j0 = jq * (JC // 2)
                        nc.tensor.matmul(
                            ps1[:, j0:j0 + JC // 2],
                            lhsT,
                            w1_sb[:, k:k + 2, j0:j0 + JC // 2],
                            start=(k == 0),
                            stop=(k == KT - 2),
                            perf_mode=DR,
                        )
                # PSUM -> SBUF: H = (64H)/64, cast to fp8 (stationary of GEMM 2)
                nc.scalar.mul(h_sb[:, m, :], ps1[:], 1.0 / WSCALE)
            for t in range(MT - 3, KT):
                at2_chunk(t)
            nc.sync.dma_start(
                atf_sb[:], atf_t.ap().rearrange("p (t j) -> p t j", j=JC))

            # ---- GEMM 2 (fp8 DoubleRow, swapped): for each 1024-col chunk mh
            #      64*x1^T[jt, m'] = H^T @ (64*AT[:, m']) ; x1 = sigmoid(/64+b1)
            #      acc[j, m'] = x1^T * W2[j] ; p2 = partition-reduce(acc)
            #      h2 chunk = AllReduce(p2) ; finals(mh) issued one chunk late.
            def finals(c):
                for i in range(NJT):
                    a = 8 * c + 2 * i
                    nc.tensor.matmul(
                        ps3[:],
                        h2b_sb[:, a:a + 2, 0:1],
                        atf_sb[:, a:a + 2, :],
                        start=(c == 0 and i == 0),
                        stop=(c == NMH - 1 and i == NJT - 1),
                        perf_mode=DR,
                    )

            ps3 = psfinal.tile([1, JC], dt.float32, tag="ps3")
            for mh in range(NMH):
                for j in range(NJT):
                    # two 512-col output chunks share each stationary load
                    ps2a = ps2pool.tile([P, MW // 2], dt.float32, tag="ps2a")
                    ps2b = ps2pool.tile([P, MW // 2], dt.float32, tag="ps2b")
                    for k in range(0, KT, 2):
                        lhsT = h_sb[:, k:k + 2, j * P:(j + 1) * P]
                        for mq, ps2 in ((0, ps2a), (1, ps2b)):
                            m0 = mh * MW + mq * (MW // 2)
                            nc.tensor.matmul(
                                ps2[:],
                                lhsT,
                                at_sb[:, k:k + 2, m0:m0 + MW // 2],
                                start=(k == 0),
                                stop=(k == KT - 2),
                                perf_mode=DR,
                            )
                    s1 = s1pool.tile([P, MW], dt.bfloat16, tag="s1")
                    for mq, ps2 in ((0, ps2a), (1, ps2b)):
                        sl = s1[:, mq * (MW // 2):(mq + 1) * (MW // 2)]
                        if with_b1:
                            nc.scalar.activation(sl, ps2[:], AF.Sigmoid,
                                                 scale=1.0 / WSCALE,
                                                 bias=b1_sb[:, j:j + 1])
                        else:
                            nc.scalar.activation(sl, ps2[:], AF.Sigmoid,
                                                 scale=1.0 / WSCALE)
                    if j == 0:
                        nc.vector.tensor_scalar(
                            out=acc_sb[:], in0=s1[:],
                            scalar1=w2_sb[:, 0:1], scalar2=None,
                            op0=ALU.mult)
                    else:
                        nc.vector.scalar_tensor_tensor(
                            out=acc_sb[:], in0=s1[:],
                            scalar=w2_sb[:, j:j + 1], in1=acc_sb[:],
                            op0=ALU.mult, op1=ALU.add)
                # p2 chunk = sum over the 128 j-partitions (all rows get it)
                nc.gpsimd.partition_all_reduce(
                    acc_sb[:], acc_sb[:], P, bass_isa.ReduceOp.add)
                nc.gpsimd.dma_start(p2d[mh][:], acc_sb[0:1, :])
                nc.gpsimd.collective_compute(
                    "AllReduce", ALU.add,
                    replica_groups=[list(range(NCORES))],
                    ins=[p2d[mh].opt()], outs=[h2d[mh].opt()])
                # transpose [8t, 128p] -> [128p, 8t] on the gather-in
                nc.gpsimd.dma_start(
                    h2g_sb[:, mh * 8:(mh + 1) * 8],
                    h2d[mh].rearrange("o (t p) -> p (o t)", p=P))
                nc.vector.tensor_copy(
                    out=h2b_sb[:, mh * 8:(mh + 1) * 8, 0:1],
                    in_=h2g_sb[:, mh * 8:(mh + 1) * 8])
                if mh >= 2:
                    finals(mh - 2)
            finals(NMH - 2)
            finals(NMH - 1)

            # out[Rc] = sigmoid(q/64 + b2)
            nc.scalar.activation(o_sb[:], ps3[:], AF.Sigmoid,
                                 scale=1.0 / WSCALE, bias=b2_sb[:])
            nc.gpsimd.dma_start(outc.ap(), o_sb[:])

    nc.compile()
    nc.m = get_hw_module(nc.m)
    return nc


def _host_preprocess(x, edge_index, W1, b1, W2, b2):
    """Build dense AT + pre-tiled fp8 operands; returns per-core in_maps."""
    edge_index = np.asarray(edge_index)
    src = edge_index[0].astype(np.int64)
    dst = edge_index[1].astype(np.int64)
    deg = np.bincount(dst, minlength=N).astype(np.float64) + 1.0
    dinv = 1.0 / np.sqrt(deg)
    vals = dinv[src] * dinv[dst]
    # AT[s, d] = A[d, s] (accumulates duplicate edges, like segment_sum)
    AT = np.bincount(src * N + dst, weights=vals, minlength=N * N)
    AT = AT.reshape(N, N)
    idx = np.arange(N)
    AT[idx, idx] += dinv * dinv
    AT64 = (AT * float(WSCALE)).astype(np.float32)

    x32 = np.asarray(x, dtype=np.float32)
    W1_32 = np.asarray(W1, dtype=np.float32)
    b1_32 = np.asarray(b1, dtype=np.float32)
    W2_32 = np.asarray(W2, dtype=np.float32).reshape(N)
    b2_32 = np.asarray(b2, dtype=np.float32).reshape(1)

    # xp_t[m, p, t*128+c] = x[m*128+c, t*128+p]
    xp_t = np.ascontiguousarray(
        x32.reshape(MT, P, KT, P).transpose(0, 3, 2, 1).reshape(MT, P, N)
    ).astype(_FP8)
    # at2_t[p, t*4096+m] = 64*AT[t*128+p, m]
    at2_t = np.ascontiguousarray(
        AT64.reshape(KT, P, N).transpose(1, 0, 2).reshape(P, KT * N)
    ).astype(_FP8)

    W1_s = (W1_32 * np.float32(WSCALE)).astype(_FP8)

    in_maps = []
    for c in range(NCORES):
        cols = slice(c * JC, (c + 1) * JC)
        w1_tc = np.ascontiguousarray(
            W1_s[:, cols].reshape(KT, P, JC).transpose(1, 0, 2).reshape(P, KT * JC)
        )
        atf_tc = np.ascontiguousarray(
            AT64[:, cols].reshape(KT, P, JC).transpose(1, 0, 2).reshape(P, KT * JC)
        ).astype(_FP8)
        in_maps.append({
            "xp_t": xp_t,
            "w1_t": w1_tc,
            "at2_t": at2_t,
            "atf_t": atf_tc,
            "w2c": np.ascontiguousarray(
                W2_32[cols].reshape(NJT, P).T).astype(np.float32),
            "b1f": np.ascontiguousarray(
                b1_32[cols].reshape(NJT, P).T).astype(np.float32),
            "b2v": b2_32.reshape(1, 1).astype(np.float32),
        })
    return in_maps


def kernel(x, edge_index, W1, b1, W2, b2, _trace=False, _premaps=None,
           _trace_cores=None):
    from concourse import bass_utils

    with_b1 = bool(np.any(np.asarray(b1)))
    key = f"nc_b1={with_b1}"
    if key not in _CACHE:
        _CACHE[key] = _build_bass_program(with_b1=with_b1)
    nc = _CACHE[key]

    in_maps = _premaps if _premaps is not None else _host_preprocess(
        x, edge_index, W1, b1, W2, b2)
    if not with_b1:
        in_maps = [{k: v for k, v in m.items() if k != "b1f"} for m in in_maps]

    res = bass_utils.run_bass_kernel_spmd(
        nc, in_maps, core_ids=list(range(NCORES)), trace=_trace,
        trace_cores=_trace_cores,
    )
    out = np.concatenate(
        [np.asarray(res.results[c]["outc"]).reshape(JC) for c in range(NCORES)]
    ).reshape(N, 1).astype(np.float32)
    if _trace:
        _CACHE["last_result"] = res
    return out
